# revision 1
# baseline (speedup 1.0000x reference)
"""Trainium2 Bass kernel for LocalScopeSelfAttention (3x3 window, clamp-padded).

Shapes (hardcoded): x [2, 8, 32, 32, 256] f32, 8 heads x hd=32, LN eps 1e-5.
Sharding: data-parallel over B*T=16 frames -> 2 frames per core on 8 cores.

Per-frame pipeline on each core (frame = 1024 tokens, D=256):
  LN (bn_stats; rstd = exp(-0.5*ln(var+eps)))
  -> xn bf16 -> DMA-transpose -> xnT (padded, 32 zero-tokens each side)
  -> projections (weights host-folded with LN affine; 1/sqrt(hd) in wq; bk
     dropped - cancels in softmax; bv folded into bo):
       qT  -> striped quad layout (zeros between head-stripes) for the
              shared-LDW block-diagonal scores matmul
       kT  -> padded f32 [d, tok]
       v   -> v_aug bf16 [tok, 8, 33] with a ones-column per head (gives the
              softmax denominator for free in the AV matmul)
  -> per 64-token subtile s (grid rows 2s, 2s+1; kctx window = 4 rows):
       scores^T [128 kctx, 8h*64q] = kT_window.T @ q_striped   (2 matmuls)
       attn_e = exp(scores^T)  (one ACT op)
       attn_m = attn_e * mask  (multiplicative {0,1,2,4} clamp-multiplicity)
       out_un[64q, 8, 33] = attn_m.T @ v_aug per head (col 32 = denominator)
       out_norm = out_un[:, :, :32] * recip(den)  -> bf16
       DMA-transpose -> xoT
  -> y = xoT.T @ wo + bo' + x  -> DMA out.
"""

import numpy as np
import ml_dtypes

H = W = 32
N = H * W          # 1024 tokens per frame
D = 256
NH, HD = 8, 32
LN_EPS = 1e-5
N_CORES = 8
FPC = 2            # frames per core
NPAD = N + 64      # padded tokens (32 guard each side)

_COMPILED = None   # (nc, input order info)


# ---------------------------------------------------------------- host helpers
def _build_masks_np():
    colcount = np.zeros((W, W), np.float32)
    for qc in range(W):
        for dc in (-1, 0, 1):
            colcount[qc, min(max(qc + dc, 0), W - 1)] += 1
    # rowcount[v][rq, rp] ; window rows are 2s-1 .. 2s+2 (rp = row - (2s-1))
    rowcounts = np.zeros((3, 2, 4), np.float32)
    for v, s in ((0, 0), (1, 7), (2, 15)):
        for rq in (0, 1):
            for dh in (-1, 0, 1):
                tgt = min(max(2 * s + rq + dh, 0), H - 1)
                rowcounts[v, rq, tgt - (2 * s - 1)] += 1
    masks = np.zeros((128, 3, 64), np.float32)
    for p in range(128):
        rp, kc = p // 32, p % 32
        for j in range(64):
            rq, qc = j // 32, j % 32
            for v in range(3):
                masks[p, v, j] = rowcounts[v, rq, rp] * colcount[qc, kc]
    return masks.astype(ml_dtypes.bfloat16)


def _fold_params(inp):
    f32 = np.float32
    g = inp["ln_g"].astype(f32)
    lb = inp["ln_b"].astype(f32)
    s = f32(1.0 / np.sqrt(HD))
    wq = (g[:, None] * inp["wq"].astype(f32)) * s
    bq = (lb @ inp["wq"].astype(f32) + inp["bq"].astype(f32)) * s
    wk = g[:, None] * inp["wk"].astype(f32)
    wv = g[:, None] * inp["wv"].astype(f32)
    bv = lb @ inp["wv"].astype(f32) + inp["bv"].astype(f32)
    wo = inp["wo"].astype(f32)
    bo = bv @ wo + inp["bo"].astype(f32)
    bf = ml_dtypes.bfloat16
    # weight sbuf layout [128, kc, m]: w[kc*128+p, m]
    def wfmt(w):
        return np.ascontiguousarray(w.reshape(2, 128, 256).transpose(1, 0, 2)).astype(bf)
    return {
        "wq": wfmt(wq), "wk": wfmt(wk), "wv": wfmt(wv), "wo": wfmt(wo),
        "bq": bq.reshape(1, 256).astype(bf),
        "bo": bo.reshape(1, 256).astype(bf),
        "masks": _build_masks_np(),
    }


# ---------------------------------------------------------------- bass build
def _build_bass():
    from contextlib import ExitStack
    import concourse.tile as tile
    from concourse import bacc, mybir

    dt = mybir.dt
    AF = mybir.ActivationFunctionType
    OP = mybir.AluOpType

    nc = bacc.Bacc("TRN2", target_bir_lowering=False, debug=False,
                   num_devices=N_CORES)

    x_d = nc.dram_tensor("x", [FPC * N, D], dt.float32, kind="ExternalInput").ap()
    wq_d = nc.dram_tensor("wq", [128, 2, 256], dt.bfloat16, kind="ExternalInput").ap()
    wk_d = nc.dram_tensor("wk", [128, 2, 256], dt.bfloat16, kind="ExternalInput").ap()
    wv_d = nc.dram_tensor("wv", [128, 2, 256], dt.bfloat16, kind="ExternalInput").ap()
    wo_d = nc.dram_tensor("wo", [128, 2, 256], dt.bfloat16, kind="ExternalInput").ap()
    bq_d = nc.dram_tensor("bq", [1, 256], dt.bfloat16, kind="ExternalInput").ap()
    bo_d = nc.dram_tensor("bo", [1, 256], dt.bfloat16, kind="ExternalInput").ap()
    mk_d = nc.dram_tensor("masks", [128, 3, 64], dt.bfloat16, kind="ExternalInput").ap()
    y_d = nc.dram_tensor("y", [FPC * N, D], dt.float32, kind="ExternalOutput").ap()

    with tile.TileContext(nc) as tc:
        with ExitStack() as ctx:
            const = ctx.enter_context(tc.tile_pool(name="const", bufs=1))
            frame = ctx.enter_context(tc.tile_pool(name="frame", bufs=1))
            work = ctx.enter_context(tc.tile_pool(name="work", bufs=3))
            att = ctx.enter_context(tc.tile_pool(name="att", bufs=3))
            pbig = ctx.enter_context(tc.tile_pool(name="pbig", bufs=1, space="PSUM"))
            pv = ctx.enter_context(tc.tile_pool(name="pv", bufs=1, space="PSUM"))
            pst = ctx.enter_context(tc.tile_pool(name="pst", bufs=2, space="PSUM"))
            pav = ctx.enter_context(tc.tile_pool(name="pav", bufs=1, space="PSUM"))

            # ---- constants ----
            wq_s = const.tile([128, 2, 256], dt.bfloat16)
            wk_s = const.tile([128, 2, 256], dt.bfloat16)
            wv_s = const.tile([128, 2, 256], dt.bfloat16)
            wo_s = const.tile([128, 2, 256], dt.bfloat16)
            for sb, d in ((wq_s, wq_d), (wk_s, wk_d), (wv_s, wv_d), (wo_s, wo_d)):
                nc.sync.dma_start(sb[:], d[:])
            bq_s = const.tile([1, 256], dt.bfloat16)
            bo_s = const.tile([1, 256], dt.bfloat16)
            mk_s = const.tile([128, 3, 64], dt.bfloat16)
            nc.sync.dma_start(bq_s[:], bq_d[:])
            nc.sync.dma_start(bo_s[:], bo_d[:])
            nc.sync.dma_start(mk_s[:], mk_d[:])
            ones_s = const.tile([1, 1024], dt.bfloat16)
            nc.vector.memset(ones_s[:], 1.0)
            ident = const.tile([128, 128], dt.bfloat16)
            from concourse.masks import make_identity
            make_identity(nc, ident[:])
            # const APs for activation biases (float bias -> AP lookup)
            for cval in (0.0, LN_EPS):
                ct = const.tile([128, 1], dt.float32, tag=f"c{cval}")
                nc.vector.memset(ct[:], cval)
                nc.const_aps.aps[(dt.float32, cval)] = ct[:]

            # ---- persistent per-frame tensors (allocated once, reused) ----
            xnT = frame.tile([128, 2, NPAD], dt.bfloat16)     # [d, kc, 32+tok]
            kTp = frame.tile([128, 2, NPAD], dt.bfloat16)
            qst = frame.tile([128, 2, 4, N], dt.bfloat16)     # striped quads
            vau = frame.tile([128, 9, NH, 33], dt.bfloat16)   # [tok%128, tok//128, h, hd+1]
            vau64 = frame.tile([128, 8, NH, 33], dt.bfloat16)  # same, tokens shifted -64
            xoT = frame.tile([128, 2, N], dt.bfloat16)
            x_f = frame.tile([128, 8, 256], dt.float32)
            mv = frame.tile([128, 8, 2], dt.float32)
            rstd = frame.tile([128, 8], dt.float32)
            lnv = frame.tile([128, 8], dt.float32)

            # zero-init pads / stripes / ones (zones not rewritten per frame)
            nc.vector.memset(xnT[:], 0.0)
            nc.vector.memset(kTp[:], 0.0)
            nc.vector.memset(qst[:], 0.0)
            nc.vector.memset(vau[:], 1.0)
            nc.vector.memset(vau64[:], 1.0)

            for f in range(FPC):
                xf_dram = x_d[f * N:(f + 1) * N, :]
                # ---------------- LN ----------------
                for i in range(8):
                    nc.sync.dma_start(x_f[:, i, :], xf_dram[128 * i:128 * (i + 1), :])
                    st = work.tile([128, 6], dt.float32, tag="bnst")
                    nc.vector.bn_stats(st[:], x_f[:, i, :])
                    nc.vector.bn_aggr(mv[:, i, :], st[:])
                # rstd = exp(-0.5*ln(var+eps))
                nc.scalar.activation(lnv[:], mv[:, :, 1], AF.Ln, bias=LN_EPS, scale=1.0)
                nc.scalar.activation(rstd[:], lnv[:], AF.Exp, bias=0.0, scale=-0.5)
                for i in range(8):
                    xn = work.tile([128, 256], dt.bfloat16, tag="xn")
                    nc.vector.tensor_scalar(
                        xn[:], x_f[:, i, :], mv[:, i, 0:1], rstd[:, i:i + 1],
                        OP.subtract, OP.mult)
                    for kc in range(2):
                        ptr = pst.tile([128, 128], dt.bfloat16, tag="pst")
                        nc.tensor.transpose(
                            ptr[:], xn[:, 128 * kc:128 * (kc + 1)], ident[:])
                        nc.scalar.copy(
                            xnT[:, kc, 32 + 128 * i:32 + 128 * (i + 1)], ptr[:])

                # ---------------- q & k projections ----------------
                for mc in range(2):
                    pq = pbig.tile([128, N], dt.float32, tag="pbig")
                    for nh in range(2):
                        ns = slice(512 * nh, 512 * (nh + 1))
                        for kc in range(2):
                            nc.tensor.matmul(
                                pq[:, ns], wq_s[:, kc, 128 * mc:128 * (mc + 1)],
                                xnT[:, kc, 32 + 512 * nh:32 + 512 * (nh + 1)],
                                start=(kc == 0), stop=False)
                        nc.tensor.matmul(
                            pq[:, ns], bq_s[0:1, 128 * mc:128 * (mc + 1)],
                            ones_s[0:1, ns],
                            start=False, stop=True)
                    qnat = work.tile([128, N], dt.bfloat16, tag="qnat")
                    nc.vector.tensor_copy(qnat[:], pq[:])
                    for g in range(4):
                        nc.sync.dma_start(
                            qst[32 * g:32 * (g + 1), mc, g, :],
                            qnat[32 * g:32 * (g + 1), :])
                for mc in range(2):
                    pk = pbig.tile([128, N], dt.float32, tag="pbig")
                    for nh in range(2):
                        ns = slice(512 * nh, 512 * (nh + 1))
                        for kc in range(2):
                            nc.tensor.matmul(
                                pk[:, ns], wk_s[:, kc, 128 * mc:128 * (mc + 1)],
                                xnT[:, kc, 32 + 512 * nh:32 + 512 * (nh + 1)],
                                start=(kc == 0), stop=(kc == 1))
                    nc.scalar.copy(kTp[:, mc, 32:32 + N], pk[:])

                # ---------------- v projection (pad-aligned chunks) ----------
                for c in range(9):
                    np_ = 128 if c < 8 else 64  # last chunk: abs tokens 1024..1087
                    pvv = pv.tile([128, NH, 33], dt.float32, tag="pv")
                    for kc in range(2):
                        nc.tensor.matmul(
                            pvv[0:np_, :, 0:32],
                            xnT[:, kc, 128 * c:128 * c + np_],
                            wv_s[:, kc, :],
                            start=(kc == 0), stop=(kc == 1))
                    nc.vector.tensor_copy(vau[0:np_, c, :, 0:32], pvv[0:np_, :, 0:32])
                # 64-shifted copy for odd subtiles (DMA can cross partitions)
                nc.sync.dma_start(vau64[0:64, :, :, :], vau[64:128, 0:8, :, :])
                nc.sync.dma_start(vau64[64:128, 0:8, :, :], vau[0:64, 1:9, :, :])

                # ---------------- attention ----------------
                for s in range(16):
                    pst_t = pst.tile([128, 2, 4, 64], dt.float32, tag="pst")
                    for Q in range(2):
                        nc.tensor.matmul(
                            pst_t[:, Q, :, :], kTp[:, Q, 64 * s:64 * s + 128],
                            qst[:, Q, :, 64 * s:64 * s + 64],
                            start=True, stop=True)
                    ae = att.tile([128, NH, 64], dt.bfloat16, tag="ae")
                    nc.scalar.activation(
                        ae[:], pst_t[:].rearrange("p q g j -> p (q g) j"),
                        AF.Exp, bias=0.0, scale=1.0)
                    am = att.tile([128, NH, 64], dt.bfloat16, tag="am")
                    vi = 0 if s == 0 else (2 if s == 15 else 1)
                    nc.gpsimd.tensor_tensor(
                        am[:], ae[:],
                        mk_s[:, vi:vi + 1, :].to_broadcast((128, NH, 64)),
                        OP.mult)
                    pa = pav.tile([64, NH, 33], dt.float32, tag="pav")
                    vsrc = vau[:, s // 2, :, :] if s % 2 == 0 else vau64[:, (s - 1) // 2, :, :]
                    for h in range(NH):
                        nc.tensor.matmul(
                            pa[:, h, :], am[:, h, :], vsrc[:, h, :],
                            start=True, stop=True)
                    rc = att.tile([64, NH], dt.float32, tag="rc")
                    nc.vector.reciprocal(rc[:], pa[:, :, 32])
                    on = att.tile([64, NH, 32], dt.bfloat16, tag="on")
                    nc.vector.tensor_tensor(
                        on[:], pa[:, :, 0:32],
                        rc[:].unsqueeze(2).to_broadcast((64, NH, 32)),
                        OP.mult)
                    onv = on[:].rearrange("p h c -> p (h c)")
                    for kc in range(2):
                        ptr = pst.tile([128, 128], dt.bfloat16, tag="pst")
                        nc.tensor.transpose(
                            ptr[0:128, 0:64], onv[:, 128 * kc:128 * (kc + 1)],
                            ident[0:64, 0:64])
                        nc.vector.tensor_copy(
                            xoT[:, kc, 64 * s:64 * (s + 1)], ptr[0:128, 0:64])

                # ---------------- out projection + residual ----------------
                for i in range(8):
                    py = pav.tile([128, 256], dt.float32, tag="py")
                    for kc in range(2):
                        nc.tensor.matmul(
                            py[:], xoT[:, kc, 128 * i:128 * (i + 1)],
                            wo_s[:, kc, :],
                            start=(kc == 0), stop=False)
                    nc.tensor.matmul(
                        py[:], ones_s[0:1, 0:128], bo_s[:],
                        start=False, stop=True)
                    ys = work.tile([128, 256], dt.float32, tag="ys")
                    nc.vector.tensor_tensor(ys[:], py[:], x_f[:, i, :], OP.add)
                    nc.sync.dma_start(
                        y_d[f * N + 128 * i:f * N + 128 * (i + 1), :], ys[:])

    nc.compile()
    return nc


# ---------------------------------------------------------------- entry point
def kernel(**inputs):
    global _COMPILED
    if _COMPILED is None:
        _COMPILED = _build_bass()
    nc = _COMPILED

    from concourse.bass_utils import run_bass_kernel_spmd

    x = np.asarray(inputs["x"], dtype=np.float32)          # [2, 8, 32, 32, 256]
    B, T = x.shape[0], x.shape[1]
    frames = x.reshape(B * T, N, D)
    params = _fold_params({k: np.asarray(v) for k, v in inputs.items()})

    in_maps = []
    for c in range(N_CORES):
        m = {"x": np.ascontiguousarray(
            frames[FPC * c:FPC * (c + 1)].reshape(FPC * N, D))}
        m.update(params)
        in_maps.append(m)

    res = run_bass_kernel_spmd(nc, in_maps, list(range(N_CORES)))
    y = np.concatenate([res.results[c]["y"].reshape(FPC, N, D)
                        for c in range(N_CORES)], axis=0)
    return y.reshape(x.shape).astype(np.float32)



# revision 6
# speedup vs baseline: 1.5639x; 1.5639x over previous
"""Trainium2 Bass kernel for LocalScopeSelfAttention (3x3 window, clamp-padded).

Shapes (hardcoded): x [2, 8, 32, 32, 256] f32, 8 heads x hd=32, LN eps 1e-5.
Sharding: data-parallel over B*T=16 frames -> 2 frames per core on 8 cores.

v2: restructured for pipeline overlap.
  - All per-frame tensors carry an explicit frame dim so the two frames'
    pipelines interleave freely (no WAR serialization on shared buffers).
  - Attention runs per 64-token subtile for scores/exp/mask, per 128-token
    pair for AV/recip/norm/transpose; the output projection for pair p is
    emitted right after pair p so PSUM rings never couple frame f+1's start
    to frame f's end.
  - The clamp-multiplicity mask multiply alternates DVE/GpSimd to balance
    engine load; exp stays on ACT reading PSUM directly.
  - PSUM: 4 tags x 2 bufs x 1-2 banks = 8 banks exactly.
"""

import numpy as np
import ml_dtypes

H = W = 32
N = H * W          # 1024 tokens per frame
D = 256
NH, HD = 8, 32
LN_EPS = 1e-5
N_CORES = 8
FPC = 2            # frames per core
NPAD = N + 64      # padded tokens (32 guard each side)

_COMPILED = None


# ---------------------------------------------------------------- host helpers
def _build_masks_np():
    colcount = np.zeros((W, W), np.float32)
    for qc in range(W):
        for dc in (-1, 0, 1):
            colcount[qc, min(max(qc + dc, 0), W - 1)] += 1
    # rowcount[v][rq, rp] ; window rows are 2s-1 .. 2s+2 (rp = row - (2s-1))
    rowcounts = np.zeros((3, 2, 4), np.float32)
    for v, s in ((0, 0), (1, 7), (2, 15)):
        for rq in (0, 1):
            for dh in (-1, 0, 1):
                tgt = min(max(2 * s + rq + dh, 0), H - 1)
                rowcounts[v, rq, tgt - (2 * s - 1)] += 1
    masks = np.zeros((128, 3, 64), np.float32)
    for p in range(128):
        rp, kc = p // 32, p % 32
        for j in range(64):
            rq, qc = j // 32, j % 32
            for v in range(3):
                masks[p, v, j] = rowcounts[v, rq, rp] * colcount[qc, kc]
    return masks.astype(ml_dtypes.bfloat16)


def _fold_params(inp):
    f32 = np.float32
    g = inp["ln_g"].astype(f32)
    lb = inp["ln_b"].astype(f32)
    s = f32(1.0 / np.sqrt(HD))
    wq = (g[:, None] * inp["wq"].astype(f32)) * s
    bq = (lb @ inp["wq"].astype(f32) + inp["bq"].astype(f32)) * s
    wk = g[:, None] * inp["wk"].astype(f32)
    wv = g[:, None] * inp["wv"].astype(f32)
    bv = lb @ inp["wv"].astype(f32) + inp["bv"].astype(f32)
    wo = inp["wo"].astype(f32)
    bo = bv @ wo + inp["bo"].astype(f32)
    bf = ml_dtypes.bfloat16
    # weight sbuf layout [128, kc, m]: w[kc*128+p, m]
    def wfmt(w):
        return np.ascontiguousarray(w.reshape(2, 128, 256).transpose(1, 0, 2)).astype(bf)
    return {
        "wq": wfmt(wq), "wk": wfmt(wk), "wv": wfmt(wv), "wo": wfmt(wo),
        "bq": bq.reshape(1, 256).astype(bf),
        "bo": bo.reshape(1, 256).astype(bf),
        "masks": _build_masks_np(),
    }


# ---------------------------------------------------------------- bass build
def _build_bass():
    from contextlib import ExitStack
    import concourse.tile as tile
    from concourse import bacc, mybir

    dt = mybir.dt
    AF = mybir.ActivationFunctionType
    OP = mybir.AluOpType

    nc = bacc.Bacc("TRN2", target_bir_lowering=False, debug=False,
                   num_devices=N_CORES)

    x_d = nc.dram_tensor("x", [FPC * N, D], dt.float32, kind="ExternalInput").ap()
    wq_d = nc.dram_tensor("wq", [128, 2, 256], dt.bfloat16, kind="ExternalInput").ap()
    wk_d = nc.dram_tensor("wk", [128, 2, 256], dt.bfloat16, kind="ExternalInput").ap()
    wv_d = nc.dram_tensor("wv", [128, 2, 256], dt.bfloat16, kind="ExternalInput").ap()
    wo_d = nc.dram_tensor("wo", [128, 2, 256], dt.bfloat16, kind="ExternalInput").ap()
    bq_d = nc.dram_tensor("bq", [1, 256], dt.bfloat16, kind="ExternalInput").ap()
    bo_d = nc.dram_tensor("bo", [1, 256], dt.bfloat16, kind="ExternalInput").ap()
    mk_d = nc.dram_tensor("masks", [128, 3, 64], dt.bfloat16, kind="ExternalInput").ap()
    y_d = nc.dram_tensor("y", [FPC * N, D], dt.float32, kind="ExternalOutput").ap()

    with tile.TileContext(nc) as tc:
        with ExitStack() as ctx:
            const = ctx.enter_context(tc.tile_pool(name="const", bufs=1))
            frame = ctx.enter_context(tc.tile_pool(name="frame", bufs=1))
            work = ctx.enter_context(tc.tile_pool(name="work", bufs=3))
            att = ctx.enter_context(tc.tile_pool(name="att", bufs=3))
            psc = ctx.enter_context(tc.tile_pool(name="psc", bufs=2, space="PSUM"))
            pav = ctx.enter_context(tc.tile_pool(name="pav", bufs=2, space="PSUM"))
            pgen = ctx.enter_context(tc.tile_pool(name="pgen", bufs=2, space="PSUM"))
            ptrp = ctx.enter_context(tc.tile_pool(name="ptrp", bufs=2, space="PSUM"))

            # ---- constants ----
            wq_s = const.tile([128, 2, 256], dt.bfloat16)
            wk_s = const.tile([128, 2, 256], dt.bfloat16)
            wv_s = const.tile([128, 2, 256], dt.bfloat16)
            wo_s = const.tile([128, 2, 256], dt.bfloat16)
            for sb, d in ((wq_s, wq_d), (wk_s, wk_d), (wv_s, wv_d), (wo_s, wo_d)):
                nc.sync.dma_start(sb[:], d[:])
            bq_s = const.tile([1, 256], dt.bfloat16)
            bo_s = const.tile([1, 256], dt.bfloat16)
            mk_s = const.tile([128, 3, 64], dt.bfloat16)
            nc.sync.dma_start(bq_s[:], bq_d[:])
            nc.sync.dma_start(bo_s[:], bo_d[:])
            nc.sync.dma_start(mk_s[:], mk_d[:])
            ones_s = const.tile([1, 1024], dt.bfloat16)
            nc.vector.memset(ones_s[:], 1.0)
            ident = const.tile([128, 128], dt.bfloat16)
            from concourse.masks import make_identity
            make_identity(nc, ident[:])
            for cval in (0.0, LN_EPS):
                ct = const.tile([128, 1], dt.float32, tag=f"c{cval}")
                nc.vector.memset(ct[:], cval)
                nc.const_aps.aps[(dt.float32, cval)] = ct[:]

            # ---- persistent per-frame tensors (frame dim f explicit) ----
            x_f = frame.tile([128, FPC, 8, 256], dt.float32)
            xnT = frame.tile([128, FPC, 2, NPAD], dt.bfloat16)
            kTp = frame.tile([128, FPC, 2, NPAD], dt.bfloat16)
            qst = frame.tile([128, FPC, 2, 4, N], dt.bfloat16)
            vau = frame.tile([128, FPC, 9, NH, 33], dt.bfloat16)
            vau64 = frame.tile([128, FPC, 8, NH, 33], dt.bfloat16)
            xoT = frame.tile([128, FPC, 2, N], dt.bfloat16)
            mv = frame.tile([128, FPC, 8, 2], dt.float32)
            rstd = frame.tile([128, FPC, 8], dt.float32)
            lnv = frame.tile([128, FPC, 8], dt.float32)

            # zero/one init of pad zones (stripes/pads are never rewritten)
            nc.vector.memset(qst[:], 0.0)
            nc.vector.memset(xnT[:], 0.0)
            nc.gpsimd.memset(kTp[:], 0.0)
            nc.gpsimd.memset(vau[:], 1.0)
            nc.gpsimd.memset(vau64[:], 1.0)

            for f in range(FPC):
                xf_dram = x_d[f * N:(f + 1) * N, :]
                # ---------------- load + LN ----------------
                nc.sync.dma_start(
                    x_f[:, f], xf_dram.rearrange("(i p) d -> p i d", p=128))
                for i in range(8):
                    st = work.tile([128, 6], dt.float32, tag="bnst")
                    nc.vector.bn_stats(st[:], x_f[:, f, i, :])
                    nc.vector.bn_aggr(mv[:, f, i, :], st[:])
                # rstd = exp(-0.5*ln(var+eps))
                nc.scalar.activation(lnv[:, f], mv[:, f, :, 1], AF.Ln,
                                     bias=LN_EPS, scale=1.0)
                nc.scalar.activation(rstd[:, f], lnv[:, f], AF.Exp,
                                     bias=0.0, scale=-0.5)
                # xn chunks + transpose into xnT (batched 2 chunks per copy)
                for u in range(4):
                    ptr = ptrp.tile([128, 4, 128], dt.bfloat16, tag="ptr")
                    for i2 in range(2):
                        i = 2 * u + i2
                        xn = work.tile([128, 256], dt.bfloat16, tag="xn")
                        nc.gpsimd.tensor_scalar(
                            xn[:], x_f[:, f, i, :], mv[:, f, i, 0:1],
                            rstd[:, f, i:i + 1], OP.subtract, OP.mult)
                        for kc in range(2):
                            nc.tensor.transpose(
                                ptr[:, 2 * i2 + kc, :],
                                xn[:, 128 * kc:128 * (kc + 1)], ident[:])
                    nc.vector.tensor_copy(
                        xnT[:, f, :, 32 + 256 * u:32 + 256 * (u + 1)]
                        .rearrange("p mc (i2 c) -> p i2 mc c", i2=2),
                        ptr[:].rearrange("p (i2 kc) c -> p i2 kc c", i2=2))

                # ---------------- q & k projections ----------------
                for mc in range(2):
                    qnat = work.tile([128, 1024], dt.bfloat16, tag="qnat")
                    for nh in range(2):
                        ns = slice(512 * nh, 512 * (nh + 1))
                        pq = pgen.tile([128, 512], dt.float32, tag="gen")
                        for kc in range(2):
                            nc.tensor.matmul(
                                pq[:], wq_s[:, kc, 128 * mc:128 * (mc + 1)],
                                xnT[:, f, kc, 32 + 512 * nh:32 + 512 * (nh + 1)],
                                start=(kc == 0), stop=False)
                        nc.tensor.matmul(
                            pq[:], bq_s[0:1, 128 * mc:128 * (mc + 1)],
                            ones_s[0:1, ns], start=False, stop=True)
                        nc.scalar.copy(qnat[:, ns], pq[:])
                    for g in range(4):
                        nc.sync.dma_start(
                            qst[32 * g:32 * (g + 1), f, mc, g, :],
                            qnat[32 * g:32 * (g + 1), :])
                for mc in range(2):
                    for nh in range(2):
                        pk = pgen.tile([128, 512], dt.float32, tag="gen")
                        for kc in range(2):
                            nc.tensor.matmul(
                                pk[:], wk_s[:, kc, 128 * mc:128 * (mc + 1)],
                                xnT[:, f, kc, 32 + 512 * nh:32 + 512 * (nh + 1)],
                                start=(kc == 0), stop=(kc == 1))
                        nc.scalar.copy(
                            kTp[:, f, mc, 32 + 512 * nh:32 + 512 * (nh + 1)],
                            pk[:])

                # ---------------- v projection (pad-aligned chunks) ----------
                for c in range(9):
                    np_ = 128 if c < 8 else 64
                    pvv = pgen.tile([128, 256], dt.float32, tag="gen")
                    for kc in range(2):
                        nc.tensor.matmul(
                            pvv[0:np_, :], xnT[:, f, kc, 128 * c:128 * c + np_],
                            wv_s[:, kc, :], start=(kc == 0), stop=(kc == 1))
                    nc.vector.tensor_copy(
                        vau[0:np_, f, c, :, 0:32],
                        pvv[0:np_, :].rearrange("p (h c) -> p h c", h=NH))
                # 64-shifted copy for odd subtiles
                nc.sync.dma_start(vau64[0:64, f], vau[64:128, f, 0:8])
                nc.sync.dma_start(vau64[64:128, f, 0:8], vau[0:64, f, 1:9])

                # ---------------- attention (+ interleaved out-proj) ---------
                for p in range(8):
                    pav_t = pav.tile([128, NH, 33], dt.float32, tag="pav")
                    for si in range(2):
                        s = 2 * p + si
                        pst = psc.tile([128, 2, 4, 64], dt.float32, tag="sc")
                        for Q in range(2):
                            nc.tensor.matmul(
                                pst[:, Q, :, :], kTp[:, f, Q, 64 * s:64 * s + 128],
                                qst[:, f, Q, :, 64 * s:64 * s + 64],
                                start=True, stop=True)
                        ae = att.tile([128, NH, 64], dt.bfloat16, tag="ae")
                        nc.scalar.activation(
                            ae[:].rearrange("p h j -> p (h j)"),
                            pst[:].rearrange("p q g j -> p (q g j)"),
                            AF.Exp, bias=0.0, scale=1.0)
                        am = att.tile([128, NH, 64], dt.bfloat16, tag="am",
                                      bufs=4)
                        vi = 0 if s == 0 else (2 if s == 15 else 1)
                        mask_ap = mk_s[:, vi:vi + 1, :].to_broadcast((128, NH, 64))
                        eng = nc.vector if s % 2 == 0 else nc.gpsimd
                        eng.tensor_tensor(am[:], ae[:], mask_ap, OP.mult)
                        vsrc = (vau[:, f, s // 2] if si == 0
                                else vau64[:, f, (s - 1) // 2])
                        for h in range(NH):
                            nc.tensor.matmul(
                                pav_t[64 * si:64 * (si + 1), h, :],
                                am[:, h, :], vsrc[:, h, :],
                                start=True, stop=True)
                    rc = att.tile([128, NH], dt.float32, tag="rc")
                    nc.vector.reciprocal(rc[:], pav_t[:, :, 32])
                    onv = att.tile([128, NH, 32], dt.bfloat16, tag="onv")
                    nc.vector.tensor_tensor(
                        onv[:], pav_t[:, :, 0:32],
                        rc[:].unsqueeze(2).to_broadcast((128, NH, 32)),
                        OP.mult)
                    onf = onv[:].rearrange("p h c -> p (h c)")
                    if p % 2 == 0:
                        ptro = pgen.tile([128, 4, 128], dt.bfloat16, tag="gen")
                    for kc in range(2):
                        nc.tensor.transpose(
                            ptro[:, 2 * (p % 2) + kc, :],
                            onf[:, 128 * kc:128 * (kc + 1)], ident[:])
                    if p % 2 == 1:
                        u = p // 2
                        nc.vector.tensor_copy(
                            xoT[:, f, :, 256 * u:256 * (u + 1)]
                            .rearrange("p mc (b c) -> p b mc c", b=2),
                            ptro[:].rearrange("p (b kc) c -> p b kc c", b=2))
                        # ---- out projection for chunks 2u, 2u+1 ----
                        for i in (2 * u, 2 * u + 1):
                            py = pgen.tile([128, 256], dt.float32, tag="gen")
                            for kc in range(2):
                                nc.tensor.matmul(
                                    py[:], xoT[:, f, kc, 128 * i:128 * (i + 1)],
                                    wo_s[:, kc, :], start=(kc == 0), stop=False)
                            nc.tensor.matmul(
                                py[:], ones_s[0:1, 0:128], bo_s[:],
                                start=False, stop=True)
                            ys = work.tile([128, 256], dt.float32, tag="ys")
                            nc.vector.tensor_tensor(
                                ys[:], py[:], x_f[:, f, i, :], OP.add)
                            nc.sync.dma_start(
                                y_d[f * N + 128 * i:f * N + 128 * (i + 1), :],
                                ys[:])

    nc.compile()
    return nc


# ---------------------------------------------------------------- entry point
def kernel(**inputs):
    global _COMPILED
    if _COMPILED is None:
        _COMPILED = _build_bass()
    nc = _COMPILED

    from concourse.bass_utils import run_bass_kernel_spmd

    x = np.asarray(inputs["x"], dtype=np.float32)          # [2, 8, 32, 32, 256]
    B, T = x.shape[0], x.shape[1]
    frames = x.reshape(B * T, N, D)
    params = _fold_params({k: np.asarray(v) for k, v in inputs.items()})

    in_maps = []
    for c in range(N_CORES):
        m = {"x": np.ascontiguousarray(
            frames[FPC * c:FPC * (c + 1)].reshape(FPC * N, D))}
        m.update(params)
        in_maps.append(m)

    res = run_bass_kernel_spmd(nc, in_maps, list(range(N_CORES)))
    y = np.concatenate([res.results[c]["y"].reshape(FPC, N, D)
                        for c in range(N_CORES)], axis=0)
    return y.reshape(x.shape).astype(np.float32)


# revision 9
# speedup vs baseline: 2.2087x; 1.4123x over previous
"""Trainium2 Bass kernel for LocalScopeSelfAttention (3x3 window, clamp-padded).

Shapes (hardcoded): x [2, 8, 32, 32, 256] f32, 8 heads x hd=32, LN eps 1e-5.
Sharding: data-parallel over B*T=16 frames -> 2 frames per core on 8 cores.

v2: restructured for pipeline overlap.
  - All per-frame tensors carry an explicit frame dim so the two frames'
    pipelines interleave freely (no WAR serialization on shared buffers).
  - Attention runs per 64-token subtile for scores/exp/mask, per 128-token
    pair for AV/recip/norm/transpose; the output projection for pair p is
    emitted right after pair p so PSUM rings never couple frame f+1's start
    to frame f's end.
  - The clamp-multiplicity mask multiply alternates DVE/GpSimd to balance
    engine load; exp stays on ACT reading PSUM directly.
  - PSUM: 4 tags x 2 bufs x 1-2 banks = 8 banks exactly.
"""

import numpy as np
import ml_dtypes

H = W = 32
N = H * W          # 1024 tokens per frame
D = 256
NH, HD = 8, 32
LN_EPS = 1e-5
N_CORES = 8
FPC = 2            # frames per core
NPAD = N + 64      # padded tokens (32 guard each side)

_COMPILED = None


# ---------------------------------------------------------------- host helpers
def _build_masks_np():
    colcount = np.zeros((W, W), np.float32)
    for qc in range(W):
        for dc in (-1, 0, 1):
            colcount[qc, min(max(qc + dc, 0), W - 1)] += 1
    # rowcount[v][rq, rp] ; window rows are 2s-1 .. 2s+2 (rp = row - (2s-1))
    rowcounts = np.zeros((3, 2, 4), np.float32)
    for v, s in ((0, 0), (1, 7), (2, 15)):
        for rq in (0, 1):
            for dh in (-1, 0, 1):
                tgt = min(max(2 * s + rq + dh, 0), H - 1)
                rowcounts[v, rq, tgt - (2 * s - 1)] += 1
    masks = np.zeros((128, 3, 64), np.float32)
    for p in range(128):
        rp, kc = p // 32, p % 32
        for j in range(64):
            rq, qc = j // 32, j % 32
            for v in range(3):
                masks[p, v, j] = rowcounts[v, rq, rp] * colcount[qc, kc]
    return masks.astype(ml_dtypes.bfloat16)


def _fold_params(inp):
    f32 = np.float32
    g = inp["ln_g"].astype(f32)
    lb = inp["ln_b"].astype(f32)
    s = f32(1.0 / np.sqrt(HD))
    wq = (g[:, None] * inp["wq"].astype(f32)) * s
    bq = (lb @ inp["wq"].astype(f32) + inp["bq"].astype(f32)) * s
    wk = g[:, None] * inp["wk"].astype(f32)
    wv = g[:, None] * inp["wv"].astype(f32)
    bv = lb @ inp["wv"].astype(f32) + inp["bv"].astype(f32)
    wo = inp["wo"].astype(f32)
    bo = bv @ wo + inp["bo"].astype(f32)
    bf = ml_dtypes.bfloat16
    # weight sbuf layout [128, kc, m]: w[kc*128+p, m]
    def wfmt(w):
        return np.ascontiguousarray(w.reshape(2, 128, 256).transpose(1, 0, 2)).astype(bf)
    return {
        "wq": wfmt(wq), "wk": wfmt(wk), "wv": wfmt(wv), "wo": wfmt(wo),
        "bq": bq.reshape(1, 256).astype(bf),
        "bo": bo.reshape(1, 256).astype(bf),
        "masks": _build_masks_np(),
    }


# ---------------------------------------------------------------- bass build
def _build_bass():
    from contextlib import ExitStack
    import concourse.tile as tile
    from concourse import bacc, mybir

    dt = mybir.dt
    AF = mybir.ActivationFunctionType
    OP = mybir.AluOpType

    nc = bacc.Bacc("TRN2", target_bir_lowering=False, debug=False,
                   num_devices=N_CORES)

    x_d = nc.dram_tensor("x", [FPC * N, D], dt.float32, kind="ExternalInput").ap()
    wq_d = nc.dram_tensor("wq", [128, 2, 256], dt.bfloat16, kind="ExternalInput").ap()
    wk_d = nc.dram_tensor("wk", [128, 2, 256], dt.bfloat16, kind="ExternalInput").ap()
    wv_d = nc.dram_tensor("wv", [128, 2, 256], dt.bfloat16, kind="ExternalInput").ap()
    wo_d = nc.dram_tensor("wo", [128, 2, 256], dt.bfloat16, kind="ExternalInput").ap()
    bq_d = nc.dram_tensor("bq", [1, 256], dt.bfloat16, kind="ExternalInput").ap()
    bo_d = nc.dram_tensor("bo", [1, 256], dt.bfloat16, kind="ExternalInput").ap()
    mk_d = nc.dram_tensor("masks", [128, 3, 64], dt.bfloat16, kind="ExternalInput").ap()
    y_d = nc.dram_tensor("y", [FPC * N, D], dt.float32, kind="ExternalOutput").ap()

    with tile.TileContext(nc) as tc:
        with ExitStack() as ctx:
            const = ctx.enter_context(tc.tile_pool(name="const", bufs=1))
            frame = ctx.enter_context(tc.tile_pool(name="frame", bufs=1))
            work = ctx.enter_context(tc.tile_pool(name="work", bufs=3))
            att = ctx.enter_context(tc.tile_pool(name="att", bufs=3))
            psc = ctx.enter_context(tc.tile_pool(name="psc", bufs=2, space="PSUM"))
            pav = ctx.enter_context(tc.tile_pool(name="pav", bufs=2, space="PSUM"))
            pgen = ctx.enter_context(tc.tile_pool(name="pgen", bufs=2, space="PSUM"))
            ptrp = ctx.enter_context(tc.tile_pool(name="ptrp", bufs=2, space="PSUM"))

            # ---- constants ----
            wq_s = const.tile([128, 2, 256], dt.bfloat16)
            wk_s = const.tile([128, 2, 256], dt.bfloat16)
            wv_s = const.tile([128, 2, 256], dt.bfloat16)
            wo_s = const.tile([128, 2, 256], dt.bfloat16)
            for sb, d in ((wq_s, wq_d), (wk_s, wk_d), (wv_s, wv_d), (wo_s, wo_d)):
                nc.sync.dma_start(sb[:], d[:])
            bq_s = const.tile([1, 256], dt.bfloat16)
            bo_s = const.tile([1, 256], dt.bfloat16)
            mk_s = const.tile([128, 3, 64], dt.bfloat16)
            nc.sync.dma_start(bq_s[:], bq_d[:])
            nc.sync.dma_start(bo_s[:], bo_d[:])
            nc.sync.dma_start(mk_s[:], mk_d[:])
            ones_s = const.tile([1, 1024], dt.bfloat16)
            nc.vector.memset(ones_s[:], 1.0)
            ident = const.tile([128, 128], dt.bfloat16)
            from concourse.masks import make_identity
            make_identity(nc, ident[:])
            for cval in (0.0, LN_EPS):
                ct = const.tile([128, 1], dt.float32, tag=f"c{cval}")
                nc.vector.memset(ct[:], cval)
                nc.const_aps.aps[(dt.float32, cval)] = ct[:]

            # ---- persistent per-frame tensors (frame dim f explicit) ----
            x_f = frame.tile([128, FPC, 8, 256], dt.float32)
            xnT = frame.tile([128, FPC, 2, NPAD], dt.bfloat16)
            kTp = frame.tile([128, FPC, 2, NPAD], dt.bfloat16)
            qst = frame.tile([128, FPC, 2, 4, N], dt.bfloat16)
            vau = frame.tile([128, FPC, 9, NH, 33], dt.bfloat16)
            vau64 = frame.tile([128, FPC, 8, NH, 33], dt.bfloat16)
            xoT = frame.tile([128, FPC, 2, N], dt.bfloat16)
            mv = frame.tile([128, FPC, 8, 2], dt.float32)
            rstd = frame.tile([128, FPC, 8], dt.float32)
            lnv = frame.tile([128, FPC, 8], dt.float32)

            # ---------------- load + LN for BOTH frames up front ----------
            # (keeps the Ln/Exp ACT table loads to one each, and lets frame 1's
            #  stats overlap frame 0's compute)
            for f in range(FPC):
                nc.sync.dma_start(
                    x_f[:, f],
                    x_d[f * N:(f + 1) * N, :].rearrange("(i p) d -> p i d", p=128))
            # zero/one init of pad zones (stripes/pads are never rewritten);
            # frame-0 halves first so frame 0's pipeline can start early
            nc.gpsimd.memset(qst[:, 0], 0.0)
            nc.vector.memset(xnT[:, 0], 0.0)
            nc.gpsimd.memset(kTp[:, 0], 0.0)
            nc.gpsimd.memset(vau[:, 0], 1.0)
            nc.gpsimd.memset(vau64[:, 0], 1.0)
            for f in range(FPC):
                for i in range(8):
                    st = work.tile([128, 6], dt.float32, tag="bnst")
                    nc.vector.bn_stats(st[:], x_f[:, f, i, :])
                    nc.vector.bn_aggr(mv[:, f, i, :], st[:])
            # rstd = exp(-0.5*ln(var+eps))
            for f in range(FPC):
                nc.scalar.activation(lnv[:, f], mv[:, f, :, 1], AF.Ln,
                                     bias=LN_EPS, scale=1.0)
            for f in range(FPC):
                nc.scalar.activation(rstd[:, f], lnv[:, f], AF.Exp,
                                     bias=0.0, scale=-0.5)

            for f in range(FPC):
                if f == 1:
                    nc.gpsimd.memset(qst[:, 1], 0.0)
                    nc.vector.memset(xnT[:, 1], 0.0)
                    nc.gpsimd.memset(kTp[:, 1], 0.0)
                    nc.gpsimd.memset(vau[:, 1], 1.0)
                    nc.gpsimd.memset(vau64[:, 1], 1.0)
                # xn chunks + transpose into xnT (batched 2 chunks per copy)
                for u in range(4):
                    ptr = ptrp.tile([128, 4, 128], dt.bfloat16, tag="ptr")
                    for i2 in range(2):
                        i = 2 * u + i2
                        xn = work.tile([128, 256], dt.bfloat16, tag="xn")
                        nc.vector.tensor_scalar(
                            xn[:], x_f[:, f, i, :], mv[:, f, i, 0:1],
                            rstd[:, f, i:i + 1], OP.subtract, OP.mult)
                        for kc in range(2):
                            nc.tensor.transpose(
                                ptr[:, 2 * i2 + kc, :],
                                xn[:, 128 * kc:128 * (kc + 1)], ident[:])
                    nc.vector.tensor_copy(
                        xnT[:, f, :, 32 + 256 * u:32 + 256 * (u + 1)]
                        .rearrange("p mc (i2 c) -> p i2 mc c", i2=2),
                        ptr[:].rearrange("p (i2 kc) c -> p i2 kc c", i2=2))

                # ---------------- q & k projections ----------------
                for mc in range(2):
                    qnat = work.tile([128, 1024], dt.bfloat16, tag="qnat")
                    for nh in range(2):
                        ns = slice(512 * nh, 512 * (nh + 1))
                        pq = pgen.tile([128, 512], dt.float32, tag="gen")
                        for kc in range(2):
                            nc.tensor.matmul(
                                pq[:], wq_s[:, kc, 128 * mc:128 * (mc + 1)],
                                xnT[:, f, kc, 32 + 512 * nh:32 + 512 * (nh + 1)],
                                start=(kc == 0), stop=False)
                        nc.tensor.matmul(
                            pq[:], bq_s[0:1, 128 * mc:128 * (mc + 1)],
                            ones_s[0:1, ns], start=False, stop=True)
                        nc.scalar.copy(qnat[:, ns], pq[:])
                    for g in range(4):
                        nc.sync.dma_start(
                            qst[32 * g:32 * (g + 1), f, mc, g, :],
                            qnat[32 * g:32 * (g + 1), :])
                for mc in range(2):
                    for nh in range(2):
                        pk = pgen.tile([128, 512], dt.float32, tag="gen")
                        for kc in range(2):
                            nc.tensor.matmul(
                                pk[:], wk_s[:, kc, 128 * mc:128 * (mc + 1)],
                                xnT[:, f, kc, 32 + 512 * nh:32 + 512 * (nh + 1)],
                                start=(kc == 0), stop=(kc == 1))
                        nc.scalar.copy(
                            kTp[:, f, mc, 32 + 512 * nh:32 + 512 * (nh + 1)],
                            pk[:])

                # ---------------- v projection (pad-aligned chunks) ----------
                for c in range(9):
                    np_ = 128 if c < 8 else 64
                    pvv = pgen.tile([128, 256], dt.float32, tag="gen")
                    for kc in range(2):
                        nc.tensor.matmul(
                            pvv[0:np_, :], xnT[:, f, kc, 128 * c:128 * c + np_],
                            wv_s[:, kc, :], start=(kc == 0), stop=(kc == 1))
                    nc.vector.tensor_copy(
                        vau[0:np_, f, c, :, 0:32],
                        pvv[0:np_, :].rearrange("p (h c) -> p h c", h=NH))
                # 64-shifted copy for odd subtiles
                nc.sync.dma_start(vau64[0:64, f], vau[64:128, f, 0:8])
                nc.sync.dma_start(vau64[64:128, f, 0:8], vau[0:64, f, 1:9])

                # ---------------- attention (+ interleaved out-proj) ---------
                for p in range(8):
                    pav_t = pav.tile([128, NH, 33], dt.float32, tag="pav")
                    for si in range(2):
                        s = 2 * p + si
                        pst = psc.tile([128, 2, 4, 64], dt.float32, tag="sc")
                        for Q in range(2):
                            nc.tensor.matmul(
                                pst[:, Q, :, :], kTp[:, f, Q, 64 * s:64 * s + 128],
                                qst[:, f, Q, :, 64 * s:64 * s + 64],
                                start=True, stop=True)
                        ae = att.tile([128, NH, 64], dt.bfloat16, tag="ae")
                        nc.scalar.activation(
                            ae[:].rearrange("p h j -> p (h j)"),
                            pst[:].rearrange("p q g j -> p (q g j)"),
                            AF.Exp, bias=0.0, scale=1.0)
                        am = att.tile([128, NH, 64], dt.bfloat16, tag="am",
                                      bufs=4)
                        vi = 0 if s == 0 else (2 if s == 15 else 1)
                        mask_ap = mk_s[:, vi:vi + 1, :].to_broadcast((128, NH, 64))
                        nc.gpsimd.tensor_tensor(am[:], ae[:], mask_ap, OP.mult)
                        vsrc = (vau[:, f, s // 2] if si == 0
                                else vau64[:, f, (s - 1) // 2])
                        for h in range(NH):
                            nc.tensor.matmul(
                                pav_t[64 * si:64 * (si + 1), h, :],
                                am[:, h, :], vsrc[:, h, :],
                                start=True, stop=True)
                    rc = att.tile([128, NH], dt.float32, tag="rc")
                    nc.vector.reciprocal(rc[:], pav_t[:, :, 32])
                    onv = att.tile([128, NH, 32], dt.bfloat16, tag="onv")
                    nc.vector.tensor_tensor(
                        onv[:], pav_t[:, :, 0:32],
                        rc[:].unsqueeze(2).to_broadcast((128, NH, 32)),
                        OP.mult)
                    onf = onv[:].rearrange("p h c -> p (h c)")
                    if p % 2 == 0:
                        ptro = pgen.tile([128, 4, 128], dt.bfloat16, tag="gen")
                    for kc in range(2):
                        nc.tensor.transpose(
                            ptro[:, 2 * (p % 2) + kc, :],
                            onf[:, 128 * kc:128 * (kc + 1)], ident[:])
                    if p % 2 == 1:
                        u = p // 2
                        nc.vector.tensor_copy(
                            xoT[:, f, :, 256 * u:256 * (u + 1)]
                            .rearrange("p mc (b c) -> p b mc c", b=2),
                            ptro[:].rearrange("p (b kc) c -> p b kc c", b=2))
                        # ---- out projection for chunks 2u, 2u+1 ----
                        for i in (2 * u, 2 * u + 1):
                            py = pgen.tile([128, 256], dt.float32, tag="gen")
                            for kc in range(2):
                                nc.tensor.matmul(
                                    py[:], xoT[:, f, kc, 128 * i:128 * (i + 1)],
                                    wo_s[:, kc, :], start=(kc == 0), stop=False)
                            nc.tensor.matmul(
                                py[:], ones_s[0:1, 0:128], bo_s[:],
                                start=False, stop=True)
                            ys = work.tile([128, 256], dt.float32, tag="ys")
                            nc.vector.tensor_tensor(
                                ys[:], py[:], x_f[:, f, i, :], OP.add)
                            nc.sync.dma_start(
                                y_d[f * N + 128 * i:f * N + 128 * (i + 1), :],
                                ys[:])

    nc.compile()
    return nc


# ---------------------------------------------------------------- entry point
def kernel(**inputs):
    global _COMPILED
    if _COMPILED is None:
        _COMPILED = _build_bass()
    nc = _COMPILED

    from concourse.bass_utils import run_bass_kernel_spmd

    x = np.asarray(inputs["x"], dtype=np.float32)          # [2, 8, 32, 32, 256]
    B, T = x.shape[0], x.shape[1]
    frames = x.reshape(B * T, N, D)
    params = _fold_params({k: np.asarray(v) for k, v in inputs.items()})

    in_maps = []
    for c in range(N_CORES):
        m = {"x": np.ascontiguousarray(
            frames[FPC * c:FPC * (c + 1)].reshape(FPC * N, D))}
        m.update(params)
        in_maps.append(m)

    res = run_bass_kernel_spmd(nc, in_maps, list(range(N_CORES)))
    y = np.concatenate([res.results[c]["y"].reshape(FPC, N, D)
                        for c in range(N_CORES)], axis=0)
    return y.reshape(x.shape).astype(np.float32)


# revision 18
# speedup vs baseline: 2.2813x; 1.0329x over previous
"""Trainium2 Bass kernel for LocalScopeSelfAttention (3x3 window, clamp-padded).

Shapes (hardcoded): x [2, 8, 32, 32, 256] f32, 8 heads x hd=32, LN eps 1e-5.
Sharding: data-parallel over B*T=16 frames -> 2 frames per core on 8 cores.

v2: restructured for pipeline overlap.
  - All per-frame tensors carry an explicit frame dim so the two frames'
    pipelines interleave freely (no WAR serialization on shared buffers).
  - Attention runs per 64-token subtile for scores/exp/mask, per 128-token
    pair for AV/recip/norm/transpose; the output projection for pair p is
    emitted right after pair p so PSUM rings never couple frame f+1's start
    to frame f's end.
  - The clamp-multiplicity mask multiply alternates DVE/GpSimd to balance
    engine load; exp stays on ACT reading PSUM directly.
  - PSUM: 4 tags x 2 bufs x 1-2 banks = 8 banks exactly.
"""

import numpy as np
import ml_dtypes

H = W = 32
N = H * W          # 1024 tokens per frame
D = 256
NH, HD = 8, 32
LN_EPS = 1e-5
N_CORES = 8
FPC = 2            # frames per core
NPAD = N + 64      # padded tokens (32 guard each side)

_COMPILED = None


# ---------------------------------------------------------------- host helpers
def _build_masks_np():
    colcount = np.zeros((W, W), np.float32)
    for qc in range(W):
        for dc in (-1, 0, 1):
            colcount[qc, min(max(qc + dc, 0), W - 1)] += 1
    # rowcount[v][rq, rp] ; window rows are 2s-1 .. 2s+2 (rp = row - (2s-1))
    rowcounts = np.zeros((3, 2, 4), np.float32)
    for v, s in ((0, 0), (1, 7), (2, 15)):
        for rq in (0, 1):
            for dh in (-1, 0, 1):
                tgt = min(max(2 * s + rq + dh, 0), H - 1)
                rowcounts[v, rq, tgt - (2 * s - 1)] += 1
    masks = np.zeros((128, 3, 64), np.float32)
    for p in range(128):
        rp, kc = p // 32, p % 32
        for j in range(64):
            rq, qc = j // 32, j % 32
            for v in range(3):
                masks[p, v, j] = rowcounts[v, rq, rp] * colcount[qc, kc]
    return masks.astype(ml_dtypes.bfloat16)


def _fold_params(inp):
    f32 = np.float32
    g = inp["ln_g"].astype(f32)
    lb = inp["ln_b"].astype(f32)
    s = f32(1.0 / np.sqrt(HD))
    wq = (g[:, None] * inp["wq"].astype(f32)) * s
    bq = (lb @ inp["wq"].astype(f32) + inp["bq"].astype(f32)) * s
    wk = g[:, None] * inp["wk"].astype(f32)
    wv = g[:, None] * inp["wv"].astype(f32)
    bv = lb @ inp["wv"].astype(f32) + inp["bv"].astype(f32)
    wo = inp["wo"].astype(f32)
    bo = bv @ wo + inp["bo"].astype(f32)
    bf = ml_dtypes.bfloat16
    # weight sbuf layout [128, kc, m]: w[kc*128+p, m]
    def wfmt(w):
        return np.ascontiguousarray(w.reshape(2, 128, 256).transpose(1, 0, 2)).astype(bf)
    return {
        "wq": wfmt(wq), "wk": wfmt(wk), "wv": wfmt(wv), "wo": wfmt(wo),
        "bq": np.ascontiguousarray(bq.reshape(2, 128).T).astype(f32),
        "bo": bo.reshape(1, 256).astype(bf),
        "masks": _build_masks_np(),
    }


# ---------------------------------------------------------------- bass build
def _build_bass():
    from contextlib import ExitStack
    import concourse.tile as tile
    from concourse import bacc, mybir

    dt = mybir.dt
    AF = mybir.ActivationFunctionType
    OP = mybir.AluOpType

    nc = bacc.Bacc("TRN2", target_bir_lowering=False, debug=False,
                   num_devices=N_CORES)

    x_d = nc.dram_tensor("x", [FPC * N, D], dt.float32, kind="ExternalInput").ap()
    wq_d = nc.dram_tensor("wq", [128, 2, 256], dt.bfloat16, kind="ExternalInput").ap()
    wk_d = nc.dram_tensor("wk", [128, 2, 256], dt.bfloat16, kind="ExternalInput").ap()
    wv_d = nc.dram_tensor("wv", [128, 2, 256], dt.bfloat16, kind="ExternalInput").ap()
    wo_d = nc.dram_tensor("wo", [128, 2, 256], dt.bfloat16, kind="ExternalInput").ap()
    bq_d = nc.dram_tensor("bq", [128, 2], dt.float32, kind="ExternalInput").ap()
    bo_d = nc.dram_tensor("bo", [1, 256], dt.bfloat16, kind="ExternalInput").ap()
    mk_d = nc.dram_tensor("masks", [128, 3, 64], dt.bfloat16, kind="ExternalInput").ap()
    y_d = nc.dram_tensor("y", [FPC * N, D], dt.float32, kind="ExternalOutput").ap()

    with tile.TileContext(nc) as tc:
        with ExitStack() as ctx:
            const = ctx.enter_context(tc.tile_pool(name="const", bufs=1))
            frame = ctx.enter_context(tc.tile_pool(name="frame", bufs=1))
            work = ctx.enter_context(tc.tile_pool(name="work", bufs=3))
            att = ctx.enter_context(tc.tile_pool(name="att", bufs=3))
            psc = ctx.enter_context(tc.tile_pool(name="psc", bufs=2, space="PSUM"))
            pav = ctx.enter_context(tc.tile_pool(name="pav", bufs=2, space="PSUM"))
            pgen = ctx.enter_context(tc.tile_pool(name="pgen", bufs=2, space="PSUM"))
            ptrp = ctx.enter_context(tc.tile_pool(name="ptrp", bufs=2, space="PSUM"))

            # ---- constants ----
            wq_s = const.tile([128, 2, 256], dt.bfloat16)
            wk_s = const.tile([128, 2, 256], dt.bfloat16)
            wv_s = const.tile([128, 2, 256], dt.bfloat16)
            wo_s = const.tile([128, 2, 256], dt.bfloat16)
            for sb, d in ((wq_s, wq_d), (wk_s, wk_d), (wv_s, wv_d), (wo_s, wo_d)):
                nc.sync.dma_start(sb[:], d[:])
            bq_s = const.tile([128, 2], dt.float32)
            bo_s = const.tile([1, 256], dt.bfloat16)
            mk_s = const.tile([128, 3, 64], dt.bfloat16)
            nc.sync.dma_start(bq_s[:], bq_d[:])
            nc.sync.dma_start(bo_s[:], bo_d[:])
            nc.sync.dma_start(mk_s[:], mk_d[:])
            ones_s = const.tile([1, 1024], dt.bfloat16)
            nc.vector.memset(ones_s[:], 1.0)
            ident = const.tile([128, 128], dt.bfloat16)
            from concourse.masks import make_identity
            make_identity(nc, ident[:])
            for cval in (0.0, LN_EPS):
                ct = const.tile([128, 1], dt.float32, tag=f"c{cval}")
                nc.vector.memset(ct[:], cval)
                nc.const_aps.aps[(dt.float32, cval)] = ct[:]

            # ---- persistent per-frame tensors (frame dim f explicit) ----
            x_f = frame.tile([128, FPC, 8, 256], dt.float32)
            xnT = frame.tile([128, FPC, 2, NPAD], dt.bfloat16)
            kTp = frame.tile([128, FPC, 2, NPAD], dt.bfloat16)
            qst = frame.tile([128, FPC, 2, 4, N], dt.bfloat16)
            vau = frame.tile([128, FPC, 9, NH, 33], dt.bfloat16)
            vau64 = frame.tile([128, FPC, 8, NH, 33], dt.bfloat16)
            xoT = frame.tile([128, FPC, 2, N], dt.bfloat16)
            ybuf = frame.tile([128, FPC, 8, 256], dt.float32)
            mv = frame.tile([128, FPC, 8, 2], dt.float32)
            rstd = frame.tile([128, FPC, 8], dt.float32)
            lnv = frame.tile([128, FPC, 8], dt.float32)

            # ---------------- load + LN for BOTH frames up front ----------
            # (keeps the Ln/Exp ACT table loads to one each, and lets frame 1's
            #  stats overlap frame 0's compute)
            for f in range(FPC):
                nc.sync.dma_start(
                    x_f[:, f],
                    x_d[f * N:(f + 1) * N, :].rearrange("(i p) d -> p i d", p=128))
            # HAM warm-up: dense PE transposes during the otherwise-idle
            # startup window so the clock gate opens before real matmuls
            warm = pgen.tile([128, 256], dt.bfloat16, tag="gen")
            for _ in range(28):
                nc.tensor.transpose(warm[:, 0:128], ident[:], ident[:])
            # zero/one init of pad zones (stripes/pads are never rewritten);
            # frame-0 halves first so frame 0's pipeline can start early
            nc.vector.memset(xnT[:, 0], 0.0)
            nc.gpsimd.memset(kTp[:, 0], 0.0)
            nc.gpsimd.memset(vau[:, 0], 1.0)
            nc.gpsimd.memset(vau64[:, 0], 1.0)
            nc.gpsimd.memset(qst[:, 0], 0.0)
            for f in range(FPC):
                for i in range(8):
                    st = work.tile([128, 6], dt.float32, tag="bnst")
                    nc.vector.bn_stats(st[:], x_f[:, f, i, :])
                    nc.vector.bn_aggr(mv[:, f, i, :], st[:])
            # rstd = exp(-0.5*ln(var+eps))
            for f in range(FPC):
                nc.scalar.activation(lnv[:, f], mv[:, f, :, 1], AF.Ln,
                                     bias=LN_EPS, scale=1.0)
            for f in range(FPC):
                nc.scalar.activation(rstd[:, f], lnv[:, f], AF.Exp,
                                     bias=0.0, scale=-0.5)

            for f in range(FPC):
                # xn chunks + transpose into xnT (batched 2 chunks per copy)
                for u in range(4):
                    ptr = ptrp.tile([128, 4, 128], dt.bfloat16, tag="ptr")
                    for i2 in range(2):
                        i = 2 * u + i2
                        xn = work.tile([128, 256], dt.bfloat16, tag="xn")
                        nc.vector.tensor_scalar(
                            xn[:], x_f[:, f, i, :], mv[:, f, i, 0:1],
                            rstd[:, f, i:i + 1], OP.subtract, OP.mult)
                        for kc in range(2):
                            nc.tensor.transpose(
                                ptr[:, 2 * i2 + kc, :],
                                xn[:, 128 * kc:128 * (kc + 1)], ident[:])
                    nc.vector.tensor_copy(
                        xnT[:, f, :, 32 + 256 * u:32 + 256 * (u + 1)]
                        .rearrange("p mc (i2 c) -> p i2 mc c", i2=2),
                        ptr[:].rearrange("p (i2 kc) c -> p i2 kc c", i2=2))

                # ---------------- q & k projections ----------------
                for mc in range(2):
                    qnat = work.tile([128, 1024], dt.bfloat16, tag="qnat")
                    for nh in range(2):
                        ns = slice(512 * nh, 512 * (nh + 1))
                        pq = pgen.tile([128, 512], dt.float32, tag="gen")
                        for kc in range(2):
                            nc.tensor.matmul(
                                pq[:], wq_s[:, kc, 128 * mc:128 * (mc + 1)],
                                xnT[:, f, kc, 32 + 512 * nh:32 + 512 * (nh + 1)],
                                start=(kc == 0), stop=(kc == 1))
                        nc.scalar.activation(qnat[:, ns], pq[:], AF.Identity,
                                             bias=bq_s[:, mc:mc + 1], scale=1.0)
                    for g in range(4):
                        nc.sync.dma_start(
                            qst[32 * g:32 * (g + 1), f, mc, g, :],
                            qnat[32 * g:32 * (g + 1), :])
                for mc in range(2):
                    for nh in range(2):
                        pk = pgen.tile([128, 512], dt.float32, tag="gen")
                        for kc in range(2):
                            nc.tensor.matmul(
                                pk[:], wk_s[:, kc, 128 * mc:128 * (mc + 1)],
                                xnT[:, f, kc, 32 + 512 * nh:32 + 512 * (nh + 1)],
                                start=(kc == 0), stop=(kc == 1))
                        nc.scalar.copy(
                            kTp[:, f, mc, 32 + 512 * nh:32 + 512 * (nh + 1)],
                            pk[:])

                # ---------------- v projection (pad-aligned chunks) ----------
                for c in range(9):
                    np_ = 128 if c < 8 else 64
                    pvv = pgen.tile([128, 256], dt.float32, tag="gen")
                    for kc in range(2):
                        nc.tensor.matmul(
                            pvv[0:np_, :], xnT[:, f, kc, 128 * c:128 * c + np_],
                            wv_s[:, kc, :], start=(kc == 0), stop=(kc == 1))
                    nc.vector.tensor_copy(
                        vau[0:np_, f, c, :, 0:32],
                        pvv[0:np_, :].rearrange("p (h c) -> p h c", h=NH))
                # 64-shifted copy for odd subtiles
                nc.sync.dma_start(vau64[0:64, f], vau[64:128, f, 0:8])
                nc.sync.dma_start(vau64[64:128, f, 0:8], vau[0:64, f, 1:9])

                if f == 0 and FPC > 1:
                    # frame-1 pad init now, while GpSimd/DVE are still light
                    nc.vector.memset(xnT[:, 1], 0.0)
                    nc.gpsimd.memset(kTp[:, 1], 0.0)
                    nc.gpsimd.memset(vau[:, 1], 1.0)
                    nc.gpsimd.memset(vau64[:, 1], 1.0)
                    nc.gpsimd.memset(qst[:, 1], 0.0)

                # ---------------- attention (+ interleaved out-proj) ---------
                for p in range(8):
                    pav_t = pav.tile([128, NH, 33], dt.float32, tag="pav")
                    for si in range(2):
                        s = 2 * p + si
                        pst = psc.tile([128, 2, 4, 64], dt.float32, tag="sc")
                        for Q in range(2):
                            nc.tensor.matmul(
                                pst[:, Q, :, :], kTp[:, f, Q, 64 * s:64 * s + 128],
                                qst[:, f, Q, :, 64 * s:64 * s + 64],
                                start=True, stop=True)
                        ae = att.tile([128, NH, 64], dt.bfloat16, tag="ae")
                        nc.scalar.activation(
                            ae[:].rearrange("p h j -> p (h j)"),
                            pst[:].rearrange("p q g j -> p (q g j)"),
                            AF.Exp, bias=0.0, scale=1.0)
                        am = att.tile([128, NH, 64], dt.bfloat16, tag="am",
                                      bufs=4)
                        vi = 0 if s == 0 else (2 if s == 15 else 1)
                        mask_ap = mk_s[:, vi:vi + 1, :].to_broadcast((128, NH, 64))
                        nc.gpsimd.tensor_tensor(am[:], ae[:], mask_ap, OP.mult)
                        vsrc = (vau[:, f, s // 2] if si == 0
                                else vau64[:, f, (s - 1) // 2])
                        for h in range(NH):
                            nc.tensor.matmul(
                                pav_t[64 * si:64 * (si + 1), h, :],
                                am[:, h, :], vsrc[:, h, :],
                                start=True, stop=True)
                    rc = att.tile([128, NH], dt.float32, tag="rc")
                    nc.vector.reciprocal(rc[:], pav_t[:, :, 32])
                    onv = att.tile([128, NH, 32], dt.bfloat16, tag="onv")
                    nc.vector.tensor_tensor(
                        onv[:], pav_t[:, :, 0:32],
                        rc[:].unsqueeze(2).to_broadcast((128, NH, 32)),
                        OP.mult)
                    onf = onv[:].rearrange("p h c -> p (h c)")
                    if p % 2 == 0:
                        ptro = pgen.tile([128, 4, 128], dt.bfloat16, tag="gen")
                    for kc in range(2):
                        nc.tensor.transpose(
                            ptro[:, 2 * (p % 2) + kc, :],
                            onf[:, 128 * kc:128 * (kc + 1)], ident[:])
                    if p % 2 == 1:
                        u = p // 2
                        nc.vector.tensor_copy(
                            xoT[:, f, :, 256 * u:256 * (u + 1)]
                            .rearrange("p mc (b c) -> p b mc c", b=2),
                            ptro[:].rearrange("p (b kc) c -> p b kc c", b=2))
                        # ---- out projection for chunks 2u, 2u+1 ----
                        for i in (2 * u, 2 * u + 1):
                            py = pgen.tile([128, 256], dt.float32, tag="gen")
                            for kc in range(2):
                                nc.tensor.matmul(
                                    py[:], xoT[:, f, kc, 128 * i:128 * (i + 1)],
                                    wo_s[:, kc, :], start=(kc == 0), stop=False)
                            nc.tensor.matmul(
                                py[:], ones_s[0:1, 0:128], bo_s[:],
                                start=False, stop=True)
                            nc.vector.tensor_tensor(
                                ybuf[:, f, i, :], py[:], x_f[:, f, i, :], OP.add)
                # one batched store per frame
                nc.sync.dma_start(
                    y_d[f * N:(f + 1) * N, :].rearrange("(i p) d -> p i d", p=128),
                    ybuf[:, f])

    nc.compile()
    return nc


# ---------------------------------------------------------------- entry point
def kernel(**inputs):
    global _COMPILED
    if _COMPILED is None:
        _COMPILED = _build_bass()
    nc = _COMPILED

    from concourse.bass_utils import run_bass_kernel_spmd

    x = np.asarray(inputs["x"], dtype=np.float32)          # [2, 8, 32, 32, 256]
    B, T = x.shape[0], x.shape[1]
    frames = x.reshape(B * T, N, D)
    params = _fold_params({k: np.asarray(v) for k, v in inputs.items()})

    in_maps = []
    for c in range(N_CORES):
        m = {"x": np.ascontiguousarray(
            frames[FPC * c:FPC * (c + 1)].reshape(FPC * N, D))}
        m.update(params)
        in_maps.append(m)

    res = run_bass_kernel_spmd(nc, in_maps, list(range(N_CORES)))
    y = np.concatenate([res.results[c]["y"].reshape(FPC, N, D)
                        for c in range(N_CORES)], axis=0)
    return y.reshape(x.shape).astype(np.float32)
